# revision 3
# baseline (speedup 1.0000x reference)
"""Trainium2 Bass kernel for NonLocalCA (embedded-gaussian non-local block on
2x2 quadrants with shared BatchNorm over the batch axis).

Problem shapes (hardcoded): x [B=2, C=64, H=128, W=128], Ci=32.
Each of the 4 quadrants is an independent 4096-token attention over both batch
elements; BatchNorm couples the two batch elements of a quadrant.

Sharding: 8 cores = 4 quadrants x 2 batch elements. Core k handles quadrant
k//2, batch k%2 and computes the full [4096, 4096] attention for its block.
The only cross-core communication is the BatchNorm (sum, sumsq) allreduce
between the two cores of a quadrant (replica groups [[0,1],[2,3],[4,5],[6,7]]).

Math per core (xf = quadrant tokens [C=64, N=4096], aug = ones row appended):
  th_rep [128, N] = TH_REP.T @ xf_aug   (4 stacked copies of theta proj + bias)
  ph_rep [128, N] = PH_REP.T @ xf_aug   (4 stacked copies of phi proj + bias)
  gxT    [N, 33]  = xf_aug.T @ G_AUG    (g proj + bias, 33rd col = ones)
  per 512-wide query block n, per 128-token key block m:
    fT[m, n-block] = ph[:, m-block].T @ th[:, n-block]      (PE, K=32)
    aT = exp(fT)                                            (ACT, PSUM->SBUF)
    yT_aug[33, n-block] += gxT[m-block].T @ aT              (PE, K=128)
  row 32 of yT_aug is the softmax denominator (ones-column trick); normalize
  after the W projection:  wy = (WT.T @ yT[0:32]) * (1/denom broadcast).
  BatchNorm stats of wy are allreduced with the sibling core, then
  out = wy*scale + (beta - mean*scale) + xf  (w_b cancels inside BN).
"""

import numpy as np

import concourse.bass as bass
import concourse.mybir as mybir
import concourse.tile as tile
from concourse import bacc
from concourse.bass_utils import run_bass_kernel_spmd

F32 = mybir.dt.float32
AF = mybir.ActivationFunctionType
ALU = mybir.AluOpType

B, C, H, W = 2, 64, 128, 128
CI = 32
HQ = H // 2  # 64
N_FULL = HQ * HQ  # 4096 tokens per quadrant
NB = 512  # query-block width (one PSUM bank of fp32)
MBLK = 128  # key-block height (partition dim)
GRP = 3  # key blocks per exp chunk (3 PSUM banks per fT tile)
BN_EPS = 1e-5


def build_nc(n_tokens=N_FULL, n_cores=8, with_collective=True, pack_mm1=True):
    """Build the SPMD Bass module. n_tokens < 4096 gives a small variant for
    simulation. Returns the compiled Bacc object."""
    NT = n_tokens
    n_nb = NT // NB  # query blocks
    n_mb = NT // MBLK  # key blocks
    bn_count = (2 if with_collective else 1) * NT

    nc = bacc.Bacc(
        "TRN2", target_bir_lowering=False, debug=False, num_devices=n_cores
    )

    xq_d = nc.dram_tensor("xq", [C, NT], F32, kind="ExternalInput")
    threp_d = nc.dram_tensor("threp", [C + 1, 128], F32, kind="ExternalInput")
    phrep_d = nc.dram_tensor("phrep", [C + 1, 128], F32, kind="ExternalInput")
    gaug_d = nc.dram_tensor("gaug", [C + 1, CI + 1], F32, kind="ExternalInput")
    wt_d = nc.dram_tensor("wt", [CI, C], F32, kind="ExternalInput")
    bnp_d = nc.dram_tensor("bnp", [C, 2], F32, kind="ExternalInput")
    out_d = nc.dram_tensor("out", [C, NT], F32, kind="ExternalOutput")
    if with_collective:
        cc_in = nc.dram_tensor("cc_in", [C, 2], F32)
        cc_out = nc.dram_tensor("cc_out", [C, 2], F32)
        groups = [[2 * q, 2 * q + 1] for q in range(n_cores // 2)]

    with tile.TileContext(nc) as tc:
        with (
            tc.tile_pool(name="consts", bufs=1) as consts,
            tc.tile_pool(name="small", bufs=2) as small,
            tc.tile_pool(name="atp", bufs=4) as atp,
            tc.tile_pool(name="outp", bufs=3) as outp,
            tc.tile_pool(name="pf", bufs=2, space="PSUM") as pf,
            tc.tile_pool(name="py", bufs=1, space="PSUM") as py,
            tc.tile_pool(name="pmisc", bufs=1, space="PSUM") as pmisc,
        ):
            # ---- load inputs / weights ----
            xf = consts.tile([C + 1, NT], F32, tag="xf")
            nc.sync.dma_start(out=xf[0:C, :], in_=xq_d[:, :])
            nc.vector.memset(xf[C : C + 1, :], 1.0)

            threp_w = consts.tile([C + 1, 128], F32, tag="threp_w")
            nc.sync.dma_start(out=threp_w, in_=threp_d[:, :])
            phrep_w = consts.tile([C + 1, 128], F32, tag="phrep_w")
            nc.sync.dma_start(out=phrep_w, in_=phrep_d[:, :])
            gaug = consts.tile([C + 1, CI + 1], F32, tag="gaug")
            nc.sync.dma_start(out=gaug, in_=gaug_d[:, :])
            wt = consts.tile([CI, C], F32, tag="wt")
            nc.sync.dma_start(out=wt, in_=wt_d[:, :])
            bnp = consts.tile([C, 2], F32, tag="bnp")
            nc.sync.dma_start(out=bnp, in_=bnp_d[:, :])

            # ---- projections ----
            # th_rep / ph_rep: [128, NT], rows 32i+j = proj row j (4 copies)
            th_rep = consts.tile([128, NT], F32, tag="th_rep")
            ph_rep = consts.tile([128, NT], F32, tag="ph_rep")
            for dst, w in ((th_rep, threp_w), (ph_rep, phrep_w)):
                for c0 in range(0, n_nb, GRP):
                    csz = min(GRP, n_nb - c0)
                    ps = pf.tile([128, GRP * NB], F32, tag="f", name="ps_proj")
                    for j in range(csz):
                        nc.tensor.matmul(
                            ps[:, j * NB : (j + 1) * NB],
                            w,
                            xf[:, (c0 + j) * NB : (c0 + j + 1) * NB],
                            start=True,
                            stop=True,
                        )
                    nc.vector.tensor_copy(
                        dst[:, c0 * NB : (c0 + csz) * NB], ps[:, : csz * NB]
                    )

            # gxT: n_mb blocks of [128, 33] packed as [128, 33*n_mb]
            gxT = consts.tile([128, (CI + 1) * n_mb], F32, tag="gxT")
            blocks_per_bank = 512 // (CI + 1)  # 15 blocks fit one fp32 bank
            m0 = 0
            while m0 < n_mb:
                bsz = min(blocks_per_bank, n_mb - m0)
                ps = pf.tile([128, GRP * NB], F32, tag="f", name="ps_gxt")
                for j in range(bsz):
                    nc.tensor.matmul(
                        ps[:, j * (CI + 1) : (j + 1) * (CI + 1)],
                        xf[:, (m0 + j) * MBLK : (m0 + j + 1) * MBLK],
                        gaug,
                        start=True,
                        stop=True,
                    )
                nc.vector.tensor_copy(
                    gxT[:, m0 * (CI + 1) : (m0 + bsz) * (CI + 1)],
                    ps[:, : bsz * (CI + 1)],
                )
                m0 += bsz

            # ---- main attention loop ----
            wy_full = consts.tile([C, NT], F32, tag="wy_full")
            s_part = consts.tile([C, n_nb], F32, tag="s_part")
            ss_part = consts.tile([C, n_nb], F32, tag="ss_part")

            for nb in range(n_nb):
                nsl = slice(nb * NB, (nb + 1) * NB)
                yps = py.tile([CI + 1, NB], F32, tag="y", name="yps")
                for g0 in range(0, n_mb, GRP):
                    gsz = min(GRP, n_mb - g0)
                    ps = pf.tile([128, GRP * NB], F32, tag="f", name="ps_f")
                    for j in range(gsz):
                        m = g0 + j
                        if pack_mm1:
                            nc.tensor.matmul(
                                ps[:, j * NB : (j + 1) * NB],
                                ph_rep[32 * j : 32 * (j + 1), m * MBLK : (m + 1) * MBLK],
                                th_rep[32 * j : 32 * (j + 1), nsl],
                                start=True,
                                stop=True,
                                tile_position=(32 * j, 0),
                            )
                        else:
                            nc.tensor.matmul(
                                ps[:, j * NB : (j + 1) * NB],
                                ph_rep[0:32, m * MBLK : (m + 1) * MBLK],
                                th_rep[0:32, nsl],
                                start=True,
                                stop=True,
                            )
                    at = atp.tile([128, GRP * NB], F32, tag="at")
                    nc.scalar.activation(at[:, : gsz * NB], ps[:, : gsz * NB], AF.Exp)
                    for j in range(gsz):
                        m = g0 + j
                        nc.tensor.matmul(
                            yps,
                            gxT[:, m * (CI + 1) : (m + 1) * (CI + 1)],
                            at[:, j * NB : (j + 1) * NB],
                            start=(m == 0),
                            stop=(m == n_mb - 1),
                        )

                # normalize + W projection
                y_sb = small.tile([CI + 1, NB], F32, tag="y_sb")
                nc.vector.tensor_copy(y_sb, yps)
                recip = small.tile([1, NB], F32, tag="recip")
                nc.vector.reciprocal(recip, y_sb[CI : CI + 1, :])
                denb = small.tile([C, NB], F32, tag="denb")
                nc.gpsimd.partition_broadcast(denb, recip)
                wyps = pmisc.tile([C, NB], F32, tag="misc", name="wyps")
                nc.tensor.matmul(wyps, wt, y_sb[0:CI, :], start=True, stop=True)
                nc.vector.tensor_mul(wy_full[:, nsl], wyps, denb)

                # BN partial stats
                sq = small.tile([C, NB], F32, tag="sq")
                nc.vector.tensor_mul(sq, wy_full[:, nsl], wy_full[:, nsl])
                nc.vector.reduce_sum(
                    out=s_part[:, nb : nb + 1], in_=wy_full[:, nsl],
                    axis=mybir.AxisListType.X,
                )
                nc.vector.reduce_sum(
                    out=ss_part[:, nb : nb + 1], in_=sq, axis=mybir.AxisListType.X
                )

            # ---- BN stats reduce (+ cross-core) ----
            stats = consts.tile([C, 2], F32, tag="stats")
            nc.vector.reduce_sum(
                out=stats[:, 0:1], in_=s_part, axis=mybir.AxisListType.X
            )
            nc.vector.reduce_sum(
                out=stats[:, 1:2], in_=ss_part, axis=mybir.AxisListType.X
            )
            if with_collective:
                nc.sync.dma_start(out=cc_in[:, :], in_=stats)
                nc.gpsimd.collective_compute(
                    "AllReduce",
                    ALU.add,
                    replica_groups=groups,
                    ins=[cc_in[:, :]],
                    outs=[cc_out[:, :]],
                )
                allstats = consts.tile([C, 2], F32, tag="allstats")
                nc.sync.dma_start(out=allstats, in_=cc_out[:, :])
            else:
                allstats = stats

            # ---- BN finalize: scale = gamma*rsqrt(var+eps), shift = beta-mean*scale
            mean_t = consts.tile([C, 1], F32, tag="mean_t")
            nc.vector.tensor_scalar_mul(mean_t, allstats[:, 0:1], 1.0 / bn_count)
            var_t = consts.tile([C, 1], F32, tag="var_t")
            nc.vector.tensor_scalar_mul(var_t, allstats[:, 1:2], 1.0 / bn_count)
            msq = consts.tile([C, 1], F32, tag="msq")
            nc.vector.tensor_mul(msq, mean_t, mean_t)
            nc.vector.tensor_sub(var_t, var_t, msq)
            # rsqrt via exp(-0.5*ln(var+eps)) — stays in the ln/exp table set
            eps_t = consts.tile([C, 1], F32, tag="eps_t")
            nc.vector.memset(eps_t, BN_EPS)
            lnv = consts.tile([C, 1], F32, tag="lnv")
            nc.scalar.activation(lnv, var_t, AF.Ln, bias=eps_t)
            rstd = consts.tile([C, 1], F32, tag="rstd")
            nc.scalar.activation(rstd, lnv, AF.Exp, scale=-0.5)
            scale_t = consts.tile([C, 1], F32, tag="scale_t")
            nc.vector.tensor_mul(scale_t, rstd, bnp[:, 0:1])
            shift_t = consts.tile([C, 1], F32, tag="shift_t")
            nc.vector.tensor_mul(shift_t, mean_t, scale_t)
            nc.vector.tensor_sub(shift_t, bnp[:, 1:2], shift_t)

            # ---- apply + residual + store ----
            for nb in range(n_nb):
                nsl = slice(nb * NB, (nb + 1) * NB)
                o_sb = outp.tile([C, NB], F32, tag="o_sb")
                nc.vector.tensor_scalar(
                    out=o_sb, in0=wy_full[:, nsl],
                    scalar1=scale_t, scalar2=shift_t,
                    op0=ALU.mult, op1=ALU.add,
                )
                nc.vector.tensor_add(o_sb, o_sb, xf[0:C, nsl])
                nc.sync.dma_start(out=out_d[:, nsl], in_=o_sb)

    nc.compile()
    return nc


def _prep_host(x, g_w, g_b, theta_w, theta_b, phi_w, phi_b, w_w, w_b,
               bn_gamma, bn_beta):
    """Host-side weight prep + input sharding. Returns (in_maps, shapes)."""
    th_aug = np.concatenate([theta_w.T, theta_b[None, :]], axis=0)  # [65, 32]
    ph_aug = np.concatenate([phi_w.T, phi_b[None, :]], axis=0)
    threp = np.tile(th_aug, (1, 4)).astype(np.float32)  # [65, 128]
    phrep = np.tile(ph_aug, (1, 4)).astype(np.float32)
    gaug = np.zeros((C + 1, CI + 1), np.float32)
    gaug[0:C, 0:CI] = g_w.T
    gaug[C, 0:CI] = g_b
    gaug[C, CI] = 1.0
    wt = np.ascontiguousarray(w_w.T).astype(np.float32)  # [32, 64]
    bnp = np.stack([bn_gamma, bn_beta], axis=1).astype(np.float32)  # [64, 2]

    in_maps = []
    for k in range(8):
        q, b = k // 2, k % 2
        qh, qw = q // 2, q % 2
        xq = x[b, :, qh * HQ : (qh + 1) * HQ, qw * HQ : (qw + 1) * HQ]
        xq = np.ascontiguousarray(xq.reshape(C, N_FULL)).astype(np.float32)
        in_maps.append(
            dict(xq=xq, threp=threp, phrep=phrep, gaug=gaug, wt=wt, bnp=bnp)
        )
    return in_maps


_NC_CACHE = {}


def _get_nc(pack_mm1=True):
    key = ("full", pack_mm1)
    if key not in _NC_CACHE:
        _NC_CACHE[key] = build_nc(
            n_tokens=N_FULL, n_cores=8, with_collective=True, pack_mm1=pack_mm1
        )
    return _NC_CACHE[key]


def kernel_with_results(trace=False, **inputs):
    """Run on 8 cores; returns (full_output [2,64,128,128], BassKernelResults)."""
    nc = _get_nc()
    in_maps = _prep_host(**inputs)
    res = run_bass_kernel_spmd(
        nc, in_maps, core_ids=list(range(8)), trace=trace
    )
    x = inputs["x"]
    out = np.empty((B, C, H, W), np.float32)
    for k in range(8):
        q, b = k // 2, k % 2
        qh, qw = q // 2, q % 2
        blk = res.results[k]["out"].reshape(C, HQ, HQ)
        out[b, :, qh * HQ : (qh + 1) * HQ, qw * HQ : (qw + 1) * HQ] = blk
    return out.astype(x.dtype), res


def kernel(**inputs):
    out, _ = kernel_with_results(trace=False, **inputs)
    return out


# revision 5
# speedup vs baseline: 1.5041x; 1.5041x over previous
"""Trainium2 Bass kernel for NonLocalCA (embedded-gaussian non-local block on
2x2 quadrants with shared BatchNorm over the batch axis).

Problem shapes (hardcoded): x [B=2, C=64, H=128, W=128], Ci=32.
Each of the 4 quadrants is an independent 4096-token attention over both batch
elements; BatchNorm couples the two batch elements of a quadrant.

Sharding: 8 cores = 4 quadrants x 2 batch elements. Core k handles quadrant
k//2, batch k%2 and computes the full [4096, 4096] attention for its block.
The only cross-core communication is the BatchNorm (sum, sumsq) allreduce
between the two cores of a quadrant (replica groups [[0,1],[2,3],[4,5],[6,7]]).

Math per core (xf = quadrant tokens [C=64, N=4096], aug = ones row appended):
  th_rep [128, N] = TH_REP.T @ xf_aug   (4 stacked copies of theta proj + bias)
  ph_rep [128, N] = PH_REP.T @ xf_aug   (4 stacked copies of phi proj + bias)
  gxT    [N, 33]  = xf_aug.T @ G_AUG    (g proj + bias, 33rd col = ones)
  per 512-wide query block n, per 128-token key block m:
    fT[m, n-block] = ph[:, m-block].T @ th[:, n-block]      (PE, K=32)
    aT = exp(fT)                                            (ACT, PSUM->SBUF)
    yT_aug[33, n-block] += gxT[m-block].T @ aT              (PE, K=128)
  row 32 of yT_aug is the softmax denominator (ones-column trick); normalize
  after the W projection:  wy = (WT.T @ yT[0:32]) * (1/denom broadcast).
  BatchNorm stats of wy are allreduced with the sibling core, then
  out = wy*scale + (beta - mean*scale) + xf  (w_b cancels inside BN).
"""

import numpy as np

import concourse.bass as bass
import concourse.mybir as mybir
import concourse.tile as tile
from concourse import bacc
from concourse.bass_utils import run_bass_kernel_spmd

F32 = mybir.dt.float32
AF = mybir.ActivationFunctionType
ALU = mybir.AluOpType

B, C, H, W = 2, 64, 128, 128
CI = 32
HQ = H // 2  # 64
N_FULL = HQ * HQ  # 4096 tokens per quadrant
NB = 512  # query-block width (one PSUM bank of fp32)
MBLK = 128  # key-block height (partition dim)
GRP = 3  # key blocks per exp chunk (3 PSUM banks per fT tile)
BN_EPS = 1e-5


def build_nc(n_tokens=N_FULL, n_cores=8, with_collective=True, pack_mm1=True):
    """Build the SPMD Bass module. n_tokens < 4096 gives a small variant for
    simulation. Returns the compiled Bacc object."""
    NT = n_tokens
    n_nb = NT // NB  # query blocks
    n_mb = NT // MBLK  # key blocks
    bn_count = (2 if with_collective else 1) * NT

    nc = bacc.Bacc(
        "TRN2", target_bir_lowering=False, debug=False, num_devices=n_cores
    )

    xq_d = nc.dram_tensor("xq", [C, NT], F32, kind="ExternalInput")
    threp_d = nc.dram_tensor("threp", [C + 1, 128], F32, kind="ExternalInput")
    phrep_d = nc.dram_tensor("phrep", [C + 1, 128], F32, kind="ExternalInput")
    gaug_d = nc.dram_tensor("gaug", [C + 1, CI + 1], F32, kind="ExternalInput")
    wt_d = nc.dram_tensor("wt", [CI, C], F32, kind="ExternalInput")
    bnp_d = nc.dram_tensor("bnp", [C, 2], F32, kind="ExternalInput")
    out_d = nc.dram_tensor("out", [C, NT], F32, kind="ExternalOutput")
    if with_collective:
        cc_in = nc.dram_tensor("cc_in", [C, 2], F32)
        cc_out = nc.dram_tensor("cc_out", [C, 2], F32)
        groups = [[2 * q, 2 * q + 1] for q in range(n_cores // 2)]

    with tile.TileContext(nc) as tc:
        with (
            tc.tile_pool(name="consts", bufs=1) as consts,
            tc.tile_pool(name="small", bufs=2) as small,
            tc.tile_pool(name="atp", bufs=4) as atp,
            tc.tile_pool(name="outp", bufs=3) as outp,
            tc.tile_pool(name="pf", bufs=2, space="PSUM") as pf,
            tc.tile_pool(name="py", bufs=2, space="PSUM") as py,
        ):
            # ---- load inputs / weights ----
            xf = consts.tile([C + 1, NT], F32, tag="xf")
            nc.sync.dma_start(out=xf[0:C, :], in_=xq_d[:, :])
            nc.vector.memset(xf[C : C + 1, :], 1.0)

            threp_w = consts.tile([C + 1, 128], F32, tag="threp_w")
            nc.sync.dma_start(out=threp_w, in_=threp_d[:, :])
            phrep_w = consts.tile([C + 1, 128], F32, tag="phrep_w")
            nc.sync.dma_start(out=phrep_w, in_=phrep_d[:, :])
            gaug = consts.tile([C + 1, CI + 1], F32, tag="gaug")
            nc.sync.dma_start(out=gaug, in_=gaug_d[:, :])
            wt = consts.tile([CI, C], F32, tag="wt")
            nc.sync.dma_start(out=wt, in_=wt_d[:, :])
            bnp = consts.tile([C, 2], F32, tag="bnp")
            nc.sync.dma_start(out=bnp, in_=bnp_d[:, :])

            # ---- projections ----
            # th_rep / ph_rep: [128, NT], rows 32i+j = proj row j (4 copies)
            th_rep = consts.tile([128, NT], F32, tag="th_rep")
            ph_rep = consts.tile([128, NT], F32, tag="ph_rep")
            for dst, w in ((th_rep, threp_w), (ph_rep, phrep_w)):
                for c0 in range(0, n_nb, GRP):
                    csz = min(GRP, n_nb - c0)
                    ps = pf.tile([128, GRP * NB], F32, tag="f", name="ps_proj")
                    for j in range(csz):
                        nc.tensor.matmul(
                            ps[:, j * NB : (j + 1) * NB],
                            w,
                            xf[:, (c0 + j) * NB : (c0 + j + 1) * NB],
                            start=True,
                            stop=True,
                        )
                    nc.vector.tensor_copy(
                        dst[:, c0 * NB : (c0 + csz) * NB], ps[:, : csz * NB]
                    )

            # gxT: n_mb blocks of [128, 33] packed as [128, 33*n_mb]
            gxT = consts.tile([128, (CI + 1) * n_mb], F32, tag="gxT")
            blocks_per_bank = 512 // (CI + 1)  # 15 blocks fit one fp32 bank
            m0 = 0
            while m0 < n_mb:
                bsz = min(blocks_per_bank, n_mb - m0)
                ps = pf.tile([128, GRP * NB], F32, tag="f", name="ps_gxt")
                for j in range(bsz):
                    nc.tensor.matmul(
                        ps[:, j * (CI + 1) : (j + 1) * (CI + 1)],
                        xf[:, (m0 + j) * MBLK : (m0 + j + 1) * MBLK],
                        gaug,
                        start=True,
                        stop=True,
                    )
                nc.vector.tensor_copy(
                    gxT[:, m0 * (CI + 1) : (m0 + bsz) * (CI + 1)],
                    ps[:, : bsz * (CI + 1)],
                )
                m0 += bsz

            # ---- main attention loop ----
            wy_full = consts.tile([C, NT], F32, tag="wy_full")
            s_part = consts.tile([C, n_nb], F32, tag="s_part")
            ss_part = consts.tile([C, n_nb], F32, tag="ss_part")

            for nb in range(n_nb):
                nsl = slice(nb * NB, (nb + 1) * NB)
                # two col-packed softmax-V accumulators (separate banks so the
                # two interleaved has_written groups don't clobber each other)
                yps_a = py.tile([128, NB], F32, tag="y", name="yps_a")
                yps_b = py.tile([128, NB], F32, tag="y", name="yps_b")
                # software pipeline: PE order is mm1(g), mm1(g+1), mm2(g), ...
                # so the PE never stalls at an mm2 waiting for exp(g).
                pending = None  # (at, g0, gsz) awaiting mm2

                def mm2_flush(pending):
                    at, g0, gsz = pending
                    for j in range(gsz):
                        m = g0 + j
                        par = m % 2
                        dst = yps_a[0:CI + 1, :] if par == 0 else yps_b[64 : 64 + CI + 1, :]
                        nc.tensor.matmul(
                            dst,
                            gxT[:, m * (CI + 1) : (m + 1) * (CI + 1)],
                            at[:, j * NB : (j + 1) * NB],
                            start=(m == par),
                            stop=(m >= n_mb - 2),
                            tile_position=(0, 64 * par),
                        )

                for g0 in range(0, n_mb, GRP):
                    gsz = min(GRP, n_mb - g0)
                    ps = pf.tile([128, GRP * NB], F32, tag="f", name="ps_f")
                    for j in range(gsz):
                        m = g0 + j
                        if pack_mm1:
                            nc.tensor.matmul(
                                ps[:, j * NB : (j + 1) * NB],
                                ph_rep[32 * j : 32 * (j + 1), m * MBLK : (m + 1) * MBLK],
                                th_rep[32 * j : 32 * (j + 1), nsl],
                                start=True,
                                stop=True,
                                tile_position=(32 * j, 0),
                            )
                        else:
                            nc.tensor.matmul(
                                ps[:, j * NB : (j + 1) * NB],
                                ph_rep[0:32, m * MBLK : (m + 1) * MBLK],
                                th_rep[0:32, nsl],
                                start=True,
                                stop=True,
                            )
                    at = atp.tile([128, GRP * NB], F32, tag="at")
                    nc.scalar.activation(at[:, : gsz * NB], ps[:, : gsz * NB], AF.Exp)
                    if pending is not None:
                        mm2_flush(pending)
                    pending = (at, g0, gsz)
                mm2_flush(pending)

                # y = stripA + stripB; normalize + W projection
                y_sb = small.tile([CI + 1, NB], F32, tag="y_sb")
                nc.vector.tensor_copy(y_sb, yps_a[0 : CI + 1, :])
                nc.vector.tensor_add(y_sb, y_sb, yps_b[64 : 64 + CI + 1, :])
                # reciprocal of the denominator row: reshape [1,NB] -> [128,NB/128]
                # through two small SBUF->SBUF DMAs so all DVE lanes participate
                rr_in = small.tile([128, NB // 128], F32, tag="rr_in")
                nc.sync.dma_start(out=rr_in, in_=y_sb[CI : CI + 1, :])
                rr4 = small.tile([128, NB // 128], F32, tag="rr4")
                nc.vector.reciprocal(rr4, rr_in)
                recip = small.tile([1, NB], F32, tag="recip")
                nc.sync.dma_start(out=recip, in_=rr4)
                denb = small.tile([C, NB], F32, tag="denb")
                nc.gpsimd.partition_broadcast(denb, recip)
                wyps = py.tile([C, NB], F32, tag="y", name="wyps")
                nc.tensor.matmul(wyps, wt, y_sb[0:CI, :], start=True, stop=True)
                nc.vector.tensor_mul(wy_full[:, nsl], wyps, denb)

                # BN partial stats
                sq = small.tile([C, NB], F32, tag="sq")
                nc.vector.tensor_mul(sq, wy_full[:, nsl], wy_full[:, nsl])
                nc.vector.reduce_sum(
                    out=s_part[:, nb : nb + 1], in_=wy_full[:, nsl],
                    axis=mybir.AxisListType.X,
                )
                nc.vector.reduce_sum(
                    out=ss_part[:, nb : nb + 1], in_=sq, axis=mybir.AxisListType.X
                )

            # ---- BN stats reduce (+ cross-core) ----
            stats = consts.tile([C, 2], F32, tag="stats")
            nc.vector.reduce_sum(
                out=stats[:, 0:1], in_=s_part, axis=mybir.AxisListType.X
            )
            nc.vector.reduce_sum(
                out=stats[:, 1:2], in_=ss_part, axis=mybir.AxisListType.X
            )
            if with_collective:
                nc.sync.dma_start(out=cc_in[:, :], in_=stats)
                nc.gpsimd.collective_compute(
                    "AllReduce",
                    ALU.add,
                    replica_groups=groups,
                    ins=[cc_in[:, :]],
                    outs=[cc_out[:, :]],
                )
                allstats = consts.tile([C, 2], F32, tag="allstats")
                nc.sync.dma_start(out=allstats, in_=cc_out[:, :])
            else:
                allstats = stats

            # ---- BN finalize: scale = gamma*rsqrt(var+eps), shift = beta-mean*scale
            mean_t = consts.tile([C, 1], F32, tag="mean_t")
            nc.vector.tensor_scalar_mul(mean_t, allstats[:, 0:1], 1.0 / bn_count)
            var_t = consts.tile([C, 1], F32, tag="var_t")
            nc.vector.tensor_scalar_mul(var_t, allstats[:, 1:2], 1.0 / bn_count)
            msq = consts.tile([C, 1], F32, tag="msq")
            nc.vector.tensor_mul(msq, mean_t, mean_t)
            nc.vector.tensor_sub(var_t, var_t, msq)
            # rsqrt via exp(-0.5*ln(var+eps)) — stays in the ln/exp table set
            eps_t = consts.tile([C, 1], F32, tag="eps_t")
            nc.vector.memset(eps_t, BN_EPS)
            lnv = consts.tile([C, 1], F32, tag="lnv")
            nc.scalar.activation(lnv, var_t, AF.Ln, bias=eps_t)
            rstd = consts.tile([C, 1], F32, tag="rstd")
            nc.scalar.activation(rstd, lnv, AF.Exp, scale=-0.5)
            scale_t = consts.tile([C, 1], F32, tag="scale_t")
            nc.vector.tensor_mul(scale_t, rstd, bnp[:, 0:1])
            shift_t = consts.tile([C, 1], F32, tag="shift_t")
            nc.vector.tensor_mul(shift_t, mean_t, scale_t)
            nc.vector.tensor_sub(shift_t, bnp[:, 1:2], shift_t)

            # ---- apply + residual + store ----
            for nb in range(n_nb):
                nsl = slice(nb * NB, (nb + 1) * NB)
                o_sb = outp.tile([C, NB], F32, tag="o_sb")
                nc.vector.tensor_scalar(
                    out=o_sb, in0=wy_full[:, nsl],
                    scalar1=scale_t, scalar2=shift_t,
                    op0=ALU.mult, op1=ALU.add,
                )
                nc.vector.tensor_add(o_sb, o_sb, xf[0:C, nsl])
                nc.sync.dma_start(out=out_d[:, nsl], in_=o_sb)

    nc.compile()
    return nc


def _prep_host(x, g_w, g_b, theta_w, theta_b, phi_w, phi_b, w_w, w_b,
               bn_gamma, bn_beta):
    """Host-side weight prep + input sharding. Returns (in_maps, shapes)."""
    th_aug = np.concatenate([theta_w.T, theta_b[None, :]], axis=0)  # [65, 32]
    ph_aug = np.concatenate([phi_w.T, phi_b[None, :]], axis=0)
    threp = np.tile(th_aug, (1, 4)).astype(np.float32)  # [65, 128]
    phrep = np.tile(ph_aug, (1, 4)).astype(np.float32)
    gaug = np.zeros((C + 1, CI + 1), np.float32)
    gaug[0:C, 0:CI] = g_w.T
    gaug[C, 0:CI] = g_b
    gaug[C, CI] = 1.0
    wt = np.ascontiguousarray(w_w.T).astype(np.float32)  # [32, 64]
    bnp = np.stack([bn_gamma, bn_beta], axis=1).astype(np.float32)  # [64, 2]

    in_maps = []
    for k in range(8):
        q, b = k // 2, k % 2
        qh, qw = q // 2, q % 2
        xq = x[b, :, qh * HQ : (qh + 1) * HQ, qw * HQ : (qw + 1) * HQ]
        xq = np.ascontiguousarray(xq.reshape(C, N_FULL)).astype(np.float32)
        in_maps.append(
            dict(xq=xq, threp=threp, phrep=phrep, gaug=gaug, wt=wt, bnp=bnp)
        )
    return in_maps


_NC_CACHE = {}


def _get_nc(pack_mm1=True):
    key = ("full", pack_mm1)
    if key not in _NC_CACHE:
        _NC_CACHE[key] = build_nc(
            n_tokens=N_FULL, n_cores=8, with_collective=True, pack_mm1=pack_mm1
        )
    return _NC_CACHE[key]


def kernel_with_results(trace=False, **inputs):
    """Run on 8 cores; returns (full_output [2,64,128,128], BassKernelResults)."""
    nc = _get_nc()
    in_maps = _prep_host(**inputs)
    res = run_bass_kernel_spmd(
        nc, in_maps, core_ids=list(range(8)), trace=trace
    )
    x = inputs["x"]
    out = np.empty((B, C, H, W), np.float32)
    for k in range(8):
        q, b = k // 2, k % 2
        qh, qw = q // 2, q % 2
        blk = res.results[k]["out"].reshape(C, HQ, HQ)
        out[b, :, qh * HQ : (qh + 1) * HQ, qw * HQ : (qw + 1) * HQ] = blk
    return out.astype(x.dtype), res


def kernel(**inputs):
    out, _ = kernel_with_results(trace=False, **inputs)
    return out
